# revision 42
# baseline (speedup 1.0000x reference)
"""Trainium2 Bass kernel for nn_LocalAtten (local attention block).

Reference computation (per sample):
  xr    = relu(conv1x1(x; w1, b1))                  # (CI=16, H, W)
  attn  = softmax(relu(conv1x1(x; w2, b2)), axis=k) # (9, H, W)
  S     = sum_k attn[k] * shift(xr, k)              # 3x3 window, zero pad
  out   = x + relu(conv1x1(S; w3, b3))              # (C=256, H, W)

Sharding: data-parallel over N; core i processes samples [2i, 2i+1].

Layout strategy (per core):
  - x is loaded channel-major: (128 c-chunk partitions, 32 h, 128 w) quarter
    tiles; these double as the residual / output staging (in-place add).
  - conv1+conv2 are fused: one matmul per image row with the x row-block as
    the STATIONARY operand (lhsT = x[c, w], rhs = W12T[c, 25]) so the output
    (w partitions, 25) is pixel-major. Biases b1/b2 are pre-filled into the
    PSUM bank by a leading K=1 ones-matmul with start=True.
  - softmax over the 9 logits is done pixel-major.  exp(relu(z)) == max(exp(z), 1)
    so the branch relu folds into a tensor_scalar_max.
  - w-shifted copies of attn (partition shifts) are made with PE matmuls
    against shifted identities (engines cannot access partition base != 0 mod 32).
  - all 9 stencil attention rows (3 w-shifts x 3 h-taps) are produced in one
    PSUM tile attC by PE matmuls against (shifted) identities; DVE reads
    attC straight from PSUM.
  - the 3x3 stencil is 3 DVE multiplies per quarter (attn broadcast via
    step-0 access patterns), writing all 9 per-(di,dj) products in the
    channel-transposed A2 layout (3dj+di, h_l, c) directly with strided
    writes -- no reformat pass and no DVE adds:
    A2[w, 3dj+di, h, c] = attC[3dj+di][w, h] * xr[w, c, h+di]
  - channel-major S is then built by PSUM-accumulating matmuls over all 9
    products: pt[j, n] += sum_w A2_k9[w, j] * I_dj[w, n], with I_dj =
    identity shifted by dj -- transpose, w-shift AND the di/dj-sum ride one
    accumulation group, with automatic zero boundaries.  Block layout:
    j = 16*h_sub + c over 8-row blocks.
  - conv3: K=128 matmuls against h_sub-selective block weight matrices
    (zero rows kill the 7 other rows packed in the block) so every operand
    stays at partition base 0 (nonzero-base matmuls fault on HW). The
    zero-padded weight tile is built on device (Pool memset + 8 tiny HBM
    loads of the compact w3 block) so the consts DMA head stays small.
    relu+b3 on ACT (per-partition bias); residual add in-place into the x
    tiles (split DVE / GpSimd to balance the tail), which are DMA'd out.
  - all tail phases run per 32-row quarter so they pipeline against later
    conv12 banks, outputs stream out early, and the freed x slots let the
    next sample's loads keep the DMA engines busy.  x tiles load q-major so
    each quarter's both c-chunks arrive together.
  - DMA queue split: x loads + oh1 stores on the (otherwise empty) SP HWDGE
    queue, oh0 stores on Pool's SWDGE queue right after Pool computed those
    residuals (data wait already satisfied), const DMAs on the ACT queue
    ahead of its compute.  Two+ live queues overlap per-DMA setup so the
    aggregate stream beats the single-queue ~330 GB/s; the kernel is
    HBM-stream-bound end to end (CoreSim 168 us/core for 67 MB + compute).
"""

import numpy as np
import ml_dtypes

import concourse.bass as bass
import concourse.bacc as bacc
import concourse.tile as tile
from concourse import mybir
from concourse.bass_utils import run_bass_kernel_spmd

F32 = mybir.dt.float32
BF16 = mybir.dt.bfloat16
AFT = mybir.ActivationFunctionType
AX = mybir.AxisListType

N_CORES = 8
NS = 2            # samples per core
C = 256
CI = 16
NK = 9
H = 128
W = 128
REG = 28          # psum col stride per row region in conv12 bank (25 used)
RPB = 16          # rows per conv12 psum bank

# packed-constant layouts
OFF_W12 = 0                     # 2 chunks x REG cols (f32)
OFF_B3 = 2 * REG                # 2 cols (f32)
CF_LEN = OFF_B3 + 2
OFF_B12 = 0                     # cf1 (partition 0 only): RPB*REG cols
OFF_ONES = OFF_B12 + RPB * REG  # 128 cols of 1.0
CF1_LEN = OFF_ONES + 128
OFF_ID = 0                      # 3 x 128 identity cols (bf16)
OFF_W3C = 3 * 128               # compact w3^T (16 rows x 2 oh x 128) (bf16)
CB_LEN = OFF_W3C + 2 * 128
# w3s SBUF tile built on-device: 8 h_sub x 2 oh x 128 cols of h_sub-
# selective block weights (rows j=16*hs+c), from 8 copies of the compact
# w3c block over a zeroed tile
W3S_LEN = 8 * 2 * 128


def _build_module():
    nc = bacc.Bacc("TRN2")
    x_d = nc.declare_dram_parameter("x", [NS, C, H, W], F32, isOutput=False)
    cf_d = nc.declare_dram_parameter("cf32", [128, CF_LEN], F32, isOutput=False)
    cf1_d = nc.declare_dram_parameter("cf1", [1, CF1_LEN], F32, isOutput=False)
    cb_d = nc.declare_dram_parameter("cbf16", [128, CB_LEN], BF16,
                                     isOutput=False)
    y_d = nc.declare_dram_parameter("y", [NS, C, H, W], F32, isOutput=True)

    from contextlib import ExitStack
    with tile.TileContext(nc) as tc, ExitStack() as ctx:
        consts = ctx.enter_context(tc.tile_pool(name="consts", bufs=1))
        xq_pool = ctx.enter_context(tc.tile_pool(name="xq", bufs=9))
        xr_pool = ctx.enter_context(tc.tile_pool(name="xr", bufs=2))
        att_pool = ctx.enter_context(tc.tile_pool(name="att", bufs=2))
        sm_pool = ctx.enter_context(tc.tile_pool(name="sm", bufs=2))
        a2_pool = ctx.enter_context(tc.tile_pool(name="a2pool", bufs=2))
        scm_pool = ctx.enter_context(tc.tile_pool(name="scm", bufs=2))
        t_pool = ctx.enter_context(tc.tile_pool(name="tst", bufs=2))
        pc12 = ctx.enter_context(tc.tile_pool(name="pc12", bufs=2, space="PSUM"))
        pT = ctx.enter_context(tc.tile_pool(name="pT", bufs=2, space="PSUM"))
        pA = ctx.enter_context(tc.tile_pool(name="pA", bufs=1, space="PSUM"))
        pwarm = ctx.enter_context(tc.tile_pool(name="pwarm", bufs=1,
                                               space="PSUM"))
        p3 = ctx.enter_context(tc.tile_pool(name="p3", bufs=2, space="PSUM"))

        # ---- constants: small packed tiles -> minimal head DMA bytes ----
        # All const DMAs are issued from the ACT queue so the SP queue can
        # start issuing the (critical-path) x loads immediately.
        cf = consts.tile([128, CF_LEN], F32)
        nc.scalar.dma_start(out=cf[:], in_=cf_d[:])
        cf1 = consts.tile([1, CF1_LEN], F32)
        nc.scalar.dma_start(out=cf1[:], in_=cf1_d[:])
        cb = consts.tile([128, CB_LEN], BF16)
        nc.scalar.dma_start(out=cb[:], in_=cb_d[:])
        # w3s (h_sub-selective zero-padded conv3 block weights) is built on
        # device: zero the tile on the idle Pool engine, then 8 tiny HBM
        # loads drop the compact w3c block on the 8 partition diagonals.
        # Issued from the ACT queue so the SP queue reaches the x loads fast.
        w3s = consts.tile([128, W3S_LEN], BF16)
        nc.gpsimd.memset(w3s[:], 0.0)
        for hs in range(8):
            nc.scalar.dma_start(
                out=w3s[16 * hs:16 * hs + 16, 256 * hs:256 * (hs + 1)],
                in_=cb_d[0:16, OFF_W3C:OFF_W3C + 256],
            )
        # tiny warm-up matmuls absorb the const-DMA waits on the PE queue so
        # no later matmul carries two sync waits (LDWEIGHTS wait-slot limit)
        warm = pwarm.tile([1, 2], F32, tag="warm")
        nc.tensor.matmul(out=warm[0:1, 0:1], lhsT=cf[0:1, 0:1],
                         rhs=cf[0:1, 0:1], start=True, stop=True)
        nc.tensor.matmul(out=warm[0:1, 1:2], lhsT=cb[0:1, 0:1],
                         rhs=cb[0:1, 0:1], start=True, stop=True)

        for s in range(NS):
            # ---- load x: 2 c-chunks x 4 h-quarters ----
            # q-major order so each quarter's both c-chunks arrive together
            # and conv12 for quarter q can start after 2(q+1) loads
            # loads all on the (DMA-only) SP queue; stores go out on the ACT
            # queue -- two live HWDGE queues overlap per-DMA setup and drive
            # the DMA engines harder than a single queue can
            xq = {}
            for q in range(4):
                for cc in range(2):
                    t = xq_pool.tile([128, 32, W], F32, tag="xq")
                    # sample 0's cc=1 loads ride Pool's SWDGE queue: at
                    # program start Pool is idle (just the w3s memset), so
                    # these 4 issues are wait-free, SP's serial stream drops
                    # to 20 transfers, and s0's quarters complete ~2x faster
                    # at the head. (s1 keeps SP: mid-program Pool is busy
                    # with residuals/stores and would issue them late.)
                    leng = nc.gpsimd if (s == 0 and cc == 1) else nc.sync
                    leng.dma_start(
                        out=t[:],
                        in_=x_d[s, cc * 128:(cc + 1) * 128, 32 * q:32 * (q + 1), :],
                    )
                    xq[(cc, q)] = t

            # pixel-major intermediates: partition = w
            xr = xr_pool.tile([128, CI, H + 2], BF16)       # (w, c, hpad)
            att = att_pool.tile([128, NK, H], BF16)         # (w, k, h)
            nc.vector.memset(xr[:, :, 0:1], 0.0)
            nc.vector.memset(xr[:, :, H + 1:H + 2], 0.0)

            # ---- conv1+conv2 fused, 8 banks of 16 rows ----
            for b in range(H // RPB):
                ps = pc12.tile([128, RPB, REG], F32, tag="ps")
                # bias pre-fill: clears has_written for the bank, writes b12
                # into every row region (start=True)
                nc.tensor.matmul(
                    out=ps[:].rearrange("p a b -> p (a b)"),
                    lhsT=cf1[0:1, OFF_ONES:OFF_ONES + 128],
                    rhs=cf1[0:1, OFF_B12:OFF_B12 + RPB * REG],
                    start=True, stop=False,
                )
                for r in range(RPB):
                    h = RPB * b + r
                    q, hl = divmod(h, 32)
                    for cc in range(2):
                        nc.tensor.matmul(
                            out=ps[:, r, 0:CI + NK],
                            lhsT=xq[(cc, q)][:, hl, :],
                            rhs=cf[:, cc * REG:cc * REG + CI + NK],
                            start=False,
                            stop=(r == RPB - 1 and cc == 1),
                        )
                # xr rows (relu): psum (128, 16r, 16c) -> xr (w, c, 1+h)
                nc.scalar.activation(
                    out=xr[:, :, 1 + RPB * b:1 + RPB * (b + 1)].transpose([0, 2, 1]),
                    in_=ps[:, :, 0:CI],
                    func=AFT.Relu,
                )
                # attention logits -> exp (relu folded in later via max(,1))
                nc.scalar.activation(
                    out=att[:, :, RPB * b:RPB * (b + 1)].transpose([0, 2, 1]),
                    in_=ps[:, :, CI:CI + NK],
                    func=AFT.Exp,
                )

            # tail phases processed per 32-row quarter so they overlap later
            # conv12 banks and release x tiles (and start output DMAs) early.
            # All quarter-local intermediates are per-quarter pool tiles
            # (bufs=2) to minimize SBUF residency.
            for g4 in range(4):
                h0 = 32 * g4
                HL = 32
                sums = sm_pool.tile([128, HL], F32, tag="sums")
                recip = sm_pool.tile([128, HL], F32, tag="recip")
                scm = scm_pool.tile([128, 4, 128], BF16)

                # ---- softmax over k (pixel-major) ----
                attv = att[:, :, h0:h0 + HL]
                nc.vector.tensor_scalar_max(out=attv, in0=attv, scalar1=1.0)
                nc.vector.reduce_sum(out=sums[:],
                                     in_=attv.transpose([0, 2, 1]), axis=AX.X)
                nc.vector.reciprocal(out=recip[:], in_=sums[:])
                nc.vector.tensor_mul(
                    out=attv, in0=attv,
                    in1=recip[:].unsqueeze(1).broadcast_to((128, NK, HL)),
                )

                # ---- all 9 stencil attn rows into PSUM via PE ----
                # attC row 3*dj + j_i = attn tap (di=j_i-1, dj) with the dj
                # w-shift applied (identity 0 = no shift, 2 = w+1, 1 = w-1).
                # Kept in PSUM; DVE reads it directly as the stencil in1.
                attC = pA.tile([128, NK, HL], F32, tag="attc")
                for r, (ident_i, k) in enumerate((
                        (0, 1), (0, 4), (0, 7),      # dj=0: ks 1,4,7
                        (2, 0), (2, 3), (2, 6),      # dj=-1: ks 0,3,6
                        (1, 2), (1, 5), (1, 8))):    # dj=+1: ks 2,5,8
                    nc.tensor.matmul(
                        out=attC[:, r, :],
                        lhsT=cb[:, OFF_ID + ident_i * 128:
                                OFF_ID + (ident_i + 1) * 128],
                        rhs=att[:, k, h0:h0 + HL],
                        start=True, stop=True,
                    )

                # ---- 3x3 stencil products, written directly in A2
                # (k9=3dj+di', h_l, c) layout (strided DVE writes do the
                # c<->h transpose). DVE only multiplies -- the di-sum rides
                # the PSUM accumulation of the transpose matmuls below.
                # A2[w, 3dj+di', h, c] = attC[3dj+di'][w, h] * xr[w, c, h+di]
                A2 = a2_pool.tile([128, NK, 32, CI], BF16, tag="a2")
                for j_i, di in enumerate((-1, 0, 1)):
                    in0_v = bass.AP(       # xr as (dj-bcast, h, c)
                        tensor=xr[:].tensor,
                        offset=xr[:].offset + 1 + h0 + di,
                        ap=[xr[:].ap[0], [0, 3], [1, HL], [H + 2, CI]],
                    )
                    in1_v = bass.AP(       # attC rows 3dj+j_i, c-bcast
                        tensor=attC[:].tensor,
                        offset=attC[:].offset + j_i * HL,
                        ap=[attC[:].ap[0], [3 * HL, 3], [1, HL], [0, CI]],
                    )
                    out_v = bass.AP(       # A2 slices 3dj+j_i
                        tensor=A2[:].tensor,
                        offset=A2[:].offset + j_i * (32 * CI),
                        ap=[A2[:].ap[0], [3 * 32 * CI, 3], [CI, HL], [1, CI]],
                    )
                    nc.vector.tensor_mul(out=out_v, in0=in0_v, in1=in1_v)
                for bl in range(4):
                    pt = pT.tile([128, 128], F32, tag="pt")
                    for k9 in range(NK):
                        lhs_v = A2[:, k9, 8 * bl:8 * (bl + 1), :] \
                            .rearrange("p a b -> p (a b)")
                        # dj group -> output shift identity (opposite of the
                        # attC pre-shift: ident 1 for dj=-1, 2 for dj=+1)
                        ident_i = k9 // 3
                        nc.tensor.matmul(
                            out=pt[:],
                            lhsT=lhs_v,
                            rhs=cb[:, OFF_ID + ident_i * 128:
                                   OFF_ID + (ident_i + 1) * 128],
                            start=(k9 == 0), stop=(k9 == NK - 1),
                        )
                    nc.scalar.copy(out=scm[:, bl, :], in_=pt[:])

                # ---- conv3 + relu(+b3) + residual add + store ----
                # K=128 matmuls with h_sub-selective block weights (zeros
                # kill the other 7 rows in the same scm block) -- base 0.
                for q in (g4,):
                    for oh in range(2):
                        for gl in range(8):
                            pp = p3.tile([128, 4, 128], F32, tag="pp")
                            for rr in range(4):
                                hq = 4 * gl + rr      # row within quarter
                                hs = (32 * q + hq) % 8
                                nc.tensor.matmul(
                                    out=pp[:, rr, :],
                                    lhsT=w3s[:, hs * 256 + oh * 128:
                                             hs * 256 + (oh + 1) * 128],
                                    rhs=scm[:, hq // 8, :],
                                    start=True, stop=True,
                                )
                            tt = t_pool.tile([128, 4, 128], F32, tag="tt")
                            nc.scalar.activation(
                                out=tt[:], in_=pp[:], func=AFT.Relu,
                                bias=cf[:, OFF_B3 + oh:OFF_B3 + oh + 1],
                                scale=1.0,
                            )
                            hl = 4 * gl
                            xv = xq[(oh, q)][:, hl:hl + 4, :]
                            # split residual adds across DVE and the otherwise
                            # idle GpSimd engine to balance the tail
                            eng = nc.gpsimd if oh == 0 else nc.vector
                            eng.tensor_add(out=xv, in0=tt[:], in1=xv)
                        # oh0 stores issue from Pool (SWDGE): Pool computed
                        # those residuals itself, so the data wait is already
                        # satisfied and a second DMA queue stays live next to
                        # SP's loads/oh1-stores. (Deferring oh1 stores onto
                        # Pool too was tried and regresses: it extends xq
                        # tile lifetimes and starves the next sample's loads.)
                        seng = nc.gpsimd if oh == 0 else nc.sync
                        seng.dma_start(
                            out=y_d[s, oh * 128:(oh + 1) * 128,
                                    32 * q:32 * (q + 1), :],
                            in_=xq[(oh, q)][:],
                        )
    nc.compile()
    return nc


_NC_CACHE = None


def _get_nc():
    global _NC_CACHE
    if _NC_CACHE is None:
        _NC_CACHE = _build_module()
    return _NC_CACHE


def _make_const_inputs(w1, b1, w2, b2, w3, b3):
    cf = np.zeros((128, CF_LEN), np.float32)
    for cc in range(2):
        cf[:, cc * REG:cc * REG + CI] = w1[:, cc * 128:(cc + 1) * 128].T
        cf[:, cc * REG + CI:cc * REG + CI + NK] = \
            w2[:, cc * 128:(cc + 1) * 128].T
    cf[:, OFF_B3:OFF_B3 + 2] = np.ascontiguousarray(b3.reshape(2, 128).T)

    cf1 = np.zeros((1, CF1_LEN), np.float32)
    b12 = np.concatenate([b1, b2]).astype(np.float32)
    for r in range(RPB):
        cf1[0, OFF_B12 + r * REG:OFF_B12 + r * REG + CI + NK] = b12
    cf1[0, OFF_ONES:OFF_ONES + 128] = 1.0

    cb = np.zeros((128, CB_LEN), np.float32)
    # idents: [0] = I (dj=0), [1] = eye(k=1) (w = n-1), [2] = eye(k=-1)
    for i, mat in enumerate((np.eye(128), np.eye(128, k=1),
                             np.eye(128, k=-1))):
        cb[:, OFF_ID + i * 128:OFF_ID + (i + 1) * 128] = mat
    # compact w3^T block (replicated on-device into the 8 w3s diagonals)
    for oh in range(2):
        cb[0:CI, OFF_W3C + oh * 128:OFF_W3C + (oh + 1) * 128] = \
            w3[oh * 128:(oh + 1) * 128, :].T
    return {"cf32": cf, "cf1": cf1, "cbf16": cb.astype(ml_dtypes.bfloat16)}


def run(x, w1, b1, w2, b2, w3, b3, trace=False):
    x = np.ascontiguousarray(np.asarray(x, dtype=np.float32))
    consts = _make_const_inputs(
        np.asarray(w1, np.float32), np.asarray(b1, np.float32),
        np.asarray(w2, np.float32), np.asarray(b2, np.float32),
        np.asarray(w3, np.float32), np.asarray(b3, np.float32))
    nc = _get_nc()
    in_maps = []
    for core in range(N_CORES):
        m = {"x": x[NS * core:NS * (core + 1)]}
        m.update(consts)
        in_maps.append(m)
    res = run_bass_kernel_spmd(nc, in_maps, list(range(N_CORES)), trace=trace)
    y = np.concatenate([res.results[i]["y"] for i in range(N_CORES)], axis=0)
    return y, res


def kernel(**inputs):
    y, _ = run(**inputs)
    return y



# revision 47
# speedup vs baseline: 1.2985x; 1.2985x over previous
"""Trainium2 Bass kernel for nn_LocalAtten (local attention block).

Reference computation (per sample):
  xr    = relu(conv1x1(x; w1, b1))                  # (CI=16, H, W)
  attn  = softmax(relu(conv1x1(x; w2, b2)), axis=k) # (9, H, W)
  S     = sum_k attn[k] * shift(xr, k)              # 3x3 window, zero pad
  out   = x + relu(conv1x1(S; w3, b3))              # (C=256, H, W)

Sharding: data-parallel over N; core i processes samples [2i, 2i+1].

Layout strategy (per core):
  - x is loaded channel-major: (128 c-chunk partitions, 32 h, 128 w) quarter
    tiles; these double as the residual / output staging (in-place add).
  - conv1+conv2 are fused: one matmul per image row with the x row-block as
    the STATIONARY operand (lhsT = x[c, w], rhs = W12T[c, 25]) so the output
    (w partitions, 25) is pixel-major. Biases b1/b2 are pre-filled into the
    PSUM bank by a leading K=1 ones-matmul with start=True.
  - softmax over the 9 logits is done pixel-major.  exp(relu(z)) == max(exp(z), 1)
    so the branch relu folds into a tensor_scalar_max.
  - w-shifted copies of attn (partition shifts) are made with PE matmuls
    against shifted identities (engines cannot access partition base != 0 mod 32).
  - all 9 stencil attention rows (3 w-shifts x 3 h-taps) are produced in one
    PSUM tile attC by PE matmuls against (shifted) identities; DVE reads
    attC straight from PSUM.
  - the 3x3 stencil is 3 DVE multiplies per quarter (attn broadcast via
    step-0 access patterns), writing all 9 per-(di,dj) products in the
    channel-transposed A2 layout (3dj+di, h_l, c) directly with strided
    writes -- no reformat pass and no DVE adds:
    A2[w, 3dj+di, h, c] = attC[3dj+di][w, h] * xr[w, c, h+di]
  - channel-major S is then built by PSUM-accumulating matmuls over all 9
    products: pt[j, n] += sum_w A2_k9[w, j] * I_dj[w, n], with I_dj =
    identity shifted by dj -- transpose, w-shift AND the di/dj-sum ride one
    accumulation group, with automatic zero boundaries.  Block layout:
    j = 16*h_sub + c over 8-row blocks.
  - conv3: K=128 matmuls against h_sub-selective block weight matrices
    (zero rows kill the 7 other rows packed in the block) so every operand
    stays at partition base 0 (nonzero-base matmuls fault on HW). The
    zero-padded weight tile is built on device (Pool memset + 8 tiny HBM
    loads of the compact w3 block) so the consts DMA head stays small.
    relu+b3 on ACT (per-partition bias); residual add in-place into the x
    tiles (split DVE / GpSimd to balance the tail), which are DMA'd out.
  - all tail phases run per 32-row quarter so they pipeline against later
    conv12 banks, outputs stream out early, and the freed x slots let the
    next sample's loads keep the DMA engines busy.  x tiles load q-major so
    each quarter's both c-chunks arrive together.
  - DMA queue split: x loads + oh1 stores on the (otherwise empty) SP HWDGE
    queue, except sample 0's cc=1 loads which ride Pool's then-idle SWDGE
    queue at program start; oh0 stores on Pool's SWDGE queue right after
    Pool computed those residuals (data wait already satisfied); const DMAs
    on the ACT queue ahead of its compute.  Three live queues overlap
    per-DMA setup so the aggregate stream beats the single-queue ~330 GB/s;
    the kernel is HBM-stream-bound end to end (CoreSim ~157 us/core for
    67 MB + compute).
"""

import numpy as np
import ml_dtypes

import concourse.bass as bass
import concourse.bacc as bacc
import concourse.tile as tile
from concourse import mybir
from concourse.bass_utils import run_bass_kernel_spmd

F32 = mybir.dt.float32
BF16 = mybir.dt.bfloat16
AFT = mybir.ActivationFunctionType
AX = mybir.AxisListType

N_CORES = 8
NS = 2            # samples per core
C = 256
CI = 16
NK = 9
H = 128
W = 128
REG = 28          # psum col stride per row region in conv12 bank (25 used)
RPB = 16          # rows per conv12 psum bank

# packed-constant layouts
OFF_W12 = 0                     # 2 chunks x REG cols (f32)
OFF_B3 = 2 * REG                # 2 cols (f32)
CF_LEN = OFF_B3 + 2
OFF_B12 = 0                     # cf1 (partition 0 only): RPB*REG cols
OFF_ONES = OFF_B12 + RPB * REG  # 128 cols of 1.0
CF1_LEN = OFF_ONES + 128
OFF_ID = 0                      # 3 x 128 identity cols (bf16)
OFF_W3C = 3 * 128               # compact w3^T (16 rows x 2 oh x 128) (bf16)
CB_LEN = OFF_W3C + 2 * 128
# w3s SBUF tile built on-device: 8 h_sub x 2 oh x 128 cols of h_sub-
# selective block weights (rows j=16*hs+c), from 8 copies of the compact
# w3c block over a zeroed tile
W3S_LEN = 8 * 2 * 128


def _build_module():
    nc = bacc.Bacc("TRN2")
    x_d = nc.declare_dram_parameter("x", [NS, C, H, W], F32, isOutput=False)
    cf_d = nc.declare_dram_parameter("cf32", [128, CF_LEN], F32, isOutput=False)
    cf1_d = nc.declare_dram_parameter("cf1", [1, CF1_LEN], F32, isOutput=False)
    cb_d = nc.declare_dram_parameter("cbf16", [128, CB_LEN], BF16,
                                     isOutput=False)
    y_d = nc.declare_dram_parameter("y", [NS, C, H, W], F32, isOutput=True)

    from contextlib import ExitStack
    with tile.TileContext(nc) as tc, ExitStack() as ctx:
        consts = ctx.enter_context(tc.tile_pool(name="consts", bufs=1))
        xq_pool = ctx.enter_context(tc.tile_pool(name="xq", bufs=10))
        xr_pool = ctx.enter_context(tc.tile_pool(name="xr", bufs=2))
        att_pool = ctx.enter_context(tc.tile_pool(name="att", bufs=2))
        sm_pool = ctx.enter_context(tc.tile_pool(name="sm", bufs=2))
        a2_pool = ctx.enter_context(tc.tile_pool(name="a2pool", bufs=2))
        scm_pool = ctx.enter_context(tc.tile_pool(name="scm", bufs=2))
        t_pool = ctx.enter_context(tc.tile_pool(name="tst", bufs=2))
        pc12 = ctx.enter_context(tc.tile_pool(name="pc12", bufs=2, space="PSUM"))
        pT = ctx.enter_context(tc.tile_pool(name="pT", bufs=2, space="PSUM"))
        pA = ctx.enter_context(tc.tile_pool(name="pA", bufs=1, space="PSUM"))
        pwarm = ctx.enter_context(tc.tile_pool(name="pwarm", bufs=1,
                                               space="PSUM"))
        p3 = ctx.enter_context(tc.tile_pool(name="p3", bufs=2, space="PSUM"))

        # ---- constants: small packed tiles -> minimal head DMA bytes ----
        # All const DMAs are issued from the ACT queue so the SP queue can
        # start issuing the (critical-path) x loads immediately.
        cf = consts.tile([128, CF_LEN], F32)
        nc.scalar.dma_start(out=cf[:], in_=cf_d[:])
        cf1 = consts.tile([1, CF1_LEN], F32)
        nc.scalar.dma_start(out=cf1[:], in_=cf1_d[:])
        cb = consts.tile([128, CB_LEN], BF16)
        nc.scalar.dma_start(out=cb[:], in_=cb_d[:])
        # w3s (h_sub-selective zero-padded conv3 block weights) is built on
        # device: zero the tile on the idle Pool engine, then 8 tiny HBM
        # loads drop the compact w3c block on the 8 partition diagonals.
        # Issued from the ACT queue so the SP queue reaches the x loads fast.
        w3s = consts.tile([128, W3S_LEN], BF16)
        nc.gpsimd.memset(w3s[:], 0.0)
        for hs in range(8):
            nc.scalar.dma_start(
                out=w3s[16 * hs:16 * hs + 16, 256 * hs:256 * (hs + 1)],
                in_=cb_d[0:16, OFF_W3C:OFF_W3C + 256],
            )
        # tiny warm-up matmuls absorb the const-DMA waits on the PE queue so
        # no later matmul carries two sync waits (LDWEIGHTS wait-slot limit)
        warm = pwarm.tile([1, 2], F32, tag="warm")
        nc.tensor.matmul(out=warm[0:1, 0:1], lhsT=cf[0:1, 0:1],
                         rhs=cf[0:1, 0:1], start=True, stop=True)
        nc.tensor.matmul(out=warm[0:1, 1:2], lhsT=cb[0:1, 0:1],
                         rhs=cb[0:1, 0:1], start=True, stop=True)

        for s in range(NS):
            # ---- load x: 2 c-chunks x 4 h-quarters ----
            # q-major order so each quarter's both c-chunks arrive together
            # and conv12 for quarter q can start after 2(q+1) loads
            # loads all on the (DMA-only) SP queue; stores go out on the ACT
            # queue -- two live HWDGE queues overlap per-DMA setup and drive
            # the DMA engines harder than a single queue can
            xq = {}
            for q in range(4):
                for cc in range(2):
                    t = xq_pool.tile([128, 32, W], F32, tag="xq")
                    # sample 0's cc=1 loads ride Pool's SWDGE queue: at
                    # program start Pool is idle (just the w3s memset), so
                    # these 4 issues are wait-free, SP's serial stream drops
                    # to 20 transfers, and s0's quarters complete ~2x faster
                    # at the head. (s1 keeps SP: mid-program Pool is busy
                    # with residuals/stores and would issue them late.)
                    leng = nc.gpsimd if (s == 0 and cc == 1) else nc.sync
                    leng.dma_start(
                        out=t[:],
                        in_=x_d[s, cc * 128:(cc + 1) * 128, 32 * q:32 * (q + 1), :],
                    )
                    xq[(cc, q)] = t

            # pixel-major intermediates: partition = w
            xr = xr_pool.tile([128, CI, H + 2], BF16)       # (w, c, hpad)
            att = att_pool.tile([128, NK, H], BF16)         # (w, k, h)
            nc.vector.memset(xr[:, :, 0:1], 0.0)
            nc.vector.memset(xr[:, :, H + 1:H + 2], 0.0)

            # ---- conv1+conv2 fused, 8 banks of 16 rows ----
            for b in range(H // RPB):
                ps = pc12.tile([128, RPB, REG], F32, tag="ps")
                # bias pre-fill: clears has_written for the bank, writes b12
                # into every row region (start=True)
                nc.tensor.matmul(
                    out=ps[:].rearrange("p a b -> p (a b)"),
                    lhsT=cf1[0:1, OFF_ONES:OFF_ONES + 128],
                    rhs=cf1[0:1, OFF_B12:OFF_B12 + RPB * REG],
                    start=True, stop=False,
                )
                for r in range(RPB):
                    h = RPB * b + r
                    q, hl = divmod(h, 32)
                    for cc in range(2):
                        nc.tensor.matmul(
                            out=ps[:, r, 0:CI + NK],
                            lhsT=xq[(cc, q)][:, hl, :],
                            rhs=cf[:, cc * REG:cc * REG + CI + NK],
                            start=False,
                            stop=(r == RPB - 1 and cc == 1),
                        )
                # xr rows (relu): psum (128, 16r, 16c) -> xr (w, c, 1+h)
                nc.scalar.activation(
                    out=xr[:, :, 1 + RPB * b:1 + RPB * (b + 1)].transpose([0, 2, 1]),
                    in_=ps[:, :, 0:CI],
                    func=AFT.Relu,
                )
                # attention logits -> exp (relu folded in later via max(,1))
                nc.scalar.activation(
                    out=att[:, :, RPB * b:RPB * (b + 1)].transpose([0, 2, 1]),
                    in_=ps[:, :, CI:CI + NK],
                    func=AFT.Exp,
                )

            # tail phases processed per 32-row quarter so they overlap later
            # conv12 banks and release x tiles (and start output DMAs) early.
            # All quarter-local intermediates are per-quarter pool tiles
            # (bufs=2) to minimize SBUF residency.
            for g4 in range(4):
                h0 = 32 * g4
                HL = 32
                sums = sm_pool.tile([128, HL], F32, tag="sums")
                recip = sm_pool.tile([128, HL], F32, tag="recip")
                scm = scm_pool.tile([128, 4, 128], BF16)

                # ---- softmax over k (pixel-major) ----
                attv = att[:, :, h0:h0 + HL]
                nc.vector.tensor_scalar_max(out=attv, in0=attv, scalar1=1.0)
                nc.vector.reduce_sum(out=sums[:],
                                     in_=attv.transpose([0, 2, 1]), axis=AX.X)
                nc.vector.reciprocal(out=recip[:], in_=sums[:])
                nc.vector.tensor_mul(
                    out=attv, in0=attv,
                    in1=recip[:].unsqueeze(1).broadcast_to((128, NK, HL)),
                )

                # ---- all 9 stencil attn rows into PSUM via PE ----
                # attC row 3*dj + j_i = attn tap (di=j_i-1, dj) with the dj
                # w-shift applied (identity 0 = no shift, 2 = w+1, 1 = w-1).
                # Kept in PSUM; DVE reads it directly as the stencil in1.
                attC = pA.tile([128, NK, HL], F32, tag="attc")
                for r, (ident_i, k) in enumerate((
                        (0, 1), (0, 4), (0, 7),      # dj=0: ks 1,4,7
                        (2, 0), (2, 3), (2, 6),      # dj=-1: ks 0,3,6
                        (1, 2), (1, 5), (1, 8))):    # dj=+1: ks 2,5,8
                    nc.tensor.matmul(
                        out=attC[:, r, :],
                        lhsT=cb[:, OFF_ID + ident_i * 128:
                                OFF_ID + (ident_i + 1) * 128],
                        rhs=att[:, k, h0:h0 + HL],
                        start=True, stop=True,
                    )

                # ---- 3x3 stencil products, written directly in A2
                # (k9=3dj+di', h_l, c) layout (strided DVE writes do the
                # c<->h transpose). DVE only multiplies -- the di-sum rides
                # the PSUM accumulation of the transpose matmuls below.
                # A2[w, 3dj+di', h, c] = attC[3dj+di'][w, h] * xr[w, c, h+di]
                A2 = a2_pool.tile([128, NK, 32, CI], BF16, tag="a2")
                for j_i, di in enumerate((-1, 0, 1)):
                    in0_v = bass.AP(       # xr as (dj-bcast, h, c)
                        tensor=xr[:].tensor,
                        offset=xr[:].offset + 1 + h0 + di,
                        ap=[xr[:].ap[0], [0, 3], [1, HL], [H + 2, CI]],
                    )
                    in1_v = bass.AP(       # attC rows 3dj+j_i, c-bcast
                        tensor=attC[:].tensor,
                        offset=attC[:].offset + j_i * HL,
                        ap=[attC[:].ap[0], [3 * HL, 3], [1, HL], [0, CI]],
                    )
                    out_v = bass.AP(       # A2 slices 3dj+j_i
                        tensor=A2[:].tensor,
                        offset=A2[:].offset + j_i * (32 * CI),
                        ap=[A2[:].ap[0], [3 * 32 * CI, 3], [CI, HL], [1, CI]],
                    )
                    nc.vector.tensor_mul(out=out_v, in0=in0_v, in1=in1_v)
                for bl in range(4):
                    pt = pT.tile([128, 128], F32, tag="pt")
                    for k9 in range(NK):
                        lhs_v = A2[:, k9, 8 * bl:8 * (bl + 1), :] \
                            .rearrange("p a b -> p (a b)")
                        # dj group -> output shift identity (opposite of the
                        # attC pre-shift: ident 1 for dj=-1, 2 for dj=+1)
                        ident_i = k9 // 3
                        nc.tensor.matmul(
                            out=pt[:],
                            lhsT=lhs_v,
                            rhs=cb[:, OFF_ID + ident_i * 128:
                                   OFF_ID + (ident_i + 1) * 128],
                            start=(k9 == 0), stop=(k9 == NK - 1),
                        )
                    nc.scalar.copy(out=scm[:, bl, :], in_=pt[:])

                # ---- conv3 + relu(+b3) + residual add + store ----
                # K=128 matmuls with h_sub-selective block weights (zeros
                # kill the other 7 rows in the same scm block) -- base 0.
                for q in (g4,):
                    for oh in range(2):
                        for gl in range(8):
                            pp = p3.tile([128, 4, 128], F32, tag="pp")
                            for rr in range(4):
                                hq = 4 * gl + rr      # row within quarter
                                hs = (32 * q + hq) % 8
                                nc.tensor.matmul(
                                    out=pp[:, rr, :],
                                    lhsT=w3s[:, hs * 256 + oh * 128:
                                             hs * 256 + (oh + 1) * 128],
                                    rhs=scm[:, hq // 8, :],
                                    start=True, stop=True,
                                )
                            tt = t_pool.tile([128, 4, 128], F32, tag="tt")
                            nc.scalar.activation(
                                out=tt[:], in_=pp[:], func=AFT.Relu,
                                bias=cf[:, OFF_B3 + oh:OFF_B3 + oh + 1],
                                scale=1.0,
                            )
                            hl = 4 * gl
                            xv = xq[(oh, q)][:, hl:hl + 4, :]
                            # split residual adds across DVE and the otherwise
                            # idle GpSimd engine to balance the tail
                            eng = nc.gpsimd if oh == 0 else nc.vector
                            eng.tensor_add(out=xv, in0=tt[:], in1=xv)
                        # oh0 stores issue from Pool (SWDGE): Pool computed
                        # those residuals itself, so the data wait is already
                        # satisfied and a second DMA queue stays live next to
                        # SP's loads/oh1-stores. (Deferring oh1 stores onto
                        # Pool too was tried and regresses: it extends xq
                        # tile lifetimes and starves the next sample's loads.)
                        seng = nc.gpsimd if oh == 0 else nc.sync
                        seng.dma_start(
                            out=y_d[s, oh * 128:(oh + 1) * 128,
                                    32 * q:32 * (q + 1), :],
                            in_=xq[(oh, q)][:],
                        )
    nc.compile()
    return nc


_NC_CACHE = None


def _get_nc():
    global _NC_CACHE
    if _NC_CACHE is None:
        _NC_CACHE = _build_module()
    return _NC_CACHE


def _make_const_inputs(w1, b1, w2, b2, w3, b3):
    cf = np.zeros((128, CF_LEN), np.float32)
    for cc in range(2):
        cf[:, cc * REG:cc * REG + CI] = w1[:, cc * 128:(cc + 1) * 128].T
        cf[:, cc * REG + CI:cc * REG + CI + NK] = \
            w2[:, cc * 128:(cc + 1) * 128].T
    cf[:, OFF_B3:OFF_B3 + 2] = np.ascontiguousarray(b3.reshape(2, 128).T)

    cf1 = np.zeros((1, CF1_LEN), np.float32)
    b12 = np.concatenate([b1, b2]).astype(np.float32)
    for r in range(RPB):
        cf1[0, OFF_B12 + r * REG:OFF_B12 + r * REG + CI + NK] = b12
    cf1[0, OFF_ONES:OFF_ONES + 128] = 1.0

    cb = np.zeros((128, CB_LEN), np.float32)
    # idents: [0] = I (dj=0), [1] = eye(k=1) (w = n-1), [2] = eye(k=-1)
    for i, mat in enumerate((np.eye(128), np.eye(128, k=1),
                             np.eye(128, k=-1))):
        cb[:, OFF_ID + i * 128:OFF_ID + (i + 1) * 128] = mat
    # compact w3^T block (replicated on-device into the 8 w3s diagonals)
    for oh in range(2):
        cb[0:CI, OFF_W3C + oh * 128:OFF_W3C + (oh + 1) * 128] = \
            w3[oh * 128:(oh + 1) * 128, :].T
    return {"cf32": cf, "cf1": cf1, "cbf16": cb.astype(ml_dtypes.bfloat16)}


def run(x, w1, b1, w2, b2, w3, b3, trace=False):
    x = np.ascontiguousarray(np.asarray(x, dtype=np.float32))
    consts = _make_const_inputs(
        np.asarray(w1, np.float32), np.asarray(b1, np.float32),
        np.asarray(w2, np.float32), np.asarray(b2, np.float32),
        np.asarray(w3, np.float32), np.asarray(b3, np.float32))
    nc = _get_nc()
    in_maps = []
    for core in range(N_CORES):
        m = {"x": x[NS * core:NS * (core + 1)]}
        m.update(consts)
        in_maps.append(m)
    res = run_bass_kernel_spmd(nc, in_maps, list(range(N_CORES)), trace=trace)
    y = np.concatenate([res.results[i]["y"] for i in range(N_CORES)], axis=0)
    return y, res


def kernel(**inputs):
    y, _ = run(**inputs)
    return y

